# revision 6
# baseline (speedup 1.0000x reference)
"""GTCN block (GCN -> temporal conv -> BN -> ReLU -> residual) on 8 TRN2 NeuronCores.

Sharding: data-parallel over samples. Each core gets 2 of the 16 samples
(30000 of the 240000 node rows); the tiny adjacency / GCN / TCN params are
replicated. No collectives.

Per-core pipeline (all matmuls bf16 inputs):
  h is pre-packed on the host (bf16, per-partition-contiguous layout
  [core, pair, u, m, j, S, c]) so the per-pair load is a single contiguous
  full-line-rate HWDGE DMA; the output is stored in the same packed layout
  and un-permuted on the host.
  Aggregation + transpose fused into ONE matmul via the stationary trick:
  sT = (bd @ h)^T = h^T bd  (bd symmetric), with the h subtile as the
  128-col stationary (FWL) and bd as the moving operand -> feature-major
  PSUM output directly; no separate PE transposes on the GCN side.
  Then gcn_w matmul (2 concurrent 64x64 quadrants) + bias + ReLU into
  xpad (128 x 7700 bf16, seq-pair on partition halves, 4*25 zero cols of
  temporal padding each side) -> 9 shifted matmuls, 4-way PE quadrant
  packing -> conv PSUM (BN scale pre-folded into the conv weights) ->
  fused bias+ReLU drain -> residual added in FEATURE-major against a
  host-packed transposed h (odd-chunk quadrant swap baked in on host, add
  runs on the otherwise-idle GpSimd engine) -> feature-major store; the
  host un-permutes. No output-side PE transposes at all.
"""

import sys

if "/opt/trn_rl_repo" not in sys.path:
    sys.path.insert(0, "/opt/trn_rl_repo")

import numpy as np
import ml_dtypes

N, M, T, V, C_IN, C_OUT, KT, PAD = 16, 2, 300, 25, 64, 64, 9, 4
BN_EPS = 1e-5

NCORES = 8
SHARD = (N // NCORES) * M * T * V      # 30000 rows per core
SEQ = T * V                            # 7500 rows per (n, m) sequence
NSEQ = 4                               # sequences per core
NPAIR = 2                              # sequence-pairs per core
NMAC = 15                              # 500-node macro tiles per sequence
MAC = 500                              # nodes per macro (20 timesteps)
SUB = 125                              # nodes per subtile (5 graphs)
NSUB = 4                               # subtiles per macro
LPAD = PAD * V                         # 100 zero cols each side of xpad
XCOLS = LPAD + SEQ + LPAD              # 7700

_BF16 = ml_dtypes.bfloat16

_CACHE = {}


def _build_nc(reps=1):
    import concourse.bass as bass
    from concourse import bacc, mybir
    from concourse.tile import TileContext
    from contextlib import ExitStack

    f32 = mybir.dt.float32
    bf16 = mybir.dt.bfloat16
    Relu = mybir.ActivationFunctionType.Relu

    nc = bacc.Bacc("TRN2", target_bir_lowering=False, debug=False)
    h_d = nc.dram_tensor(
        "h", [NPAIR * SUB, NMAC * NSUB * 2 * C_IN], bf16, kind="ExternalInput"
    )
    bd_d = nc.dram_tensor("bd", [SUB, 128], bf16, kind="ExternalInput")
    gw_d = nc.dram_tensor("gw", [128, C_OUT], bf16, kind="ExternalInput")
    cw_d = nc.dram_tensor("cw", [128, KT * C_OUT], bf16, kind="ExternalInput")
    gb_d = nc.dram_tensor("gb", [128, 1], f32, kind="ExternalInput")
    bnb_d = nc.dram_tensor("bnb", [128, 1], f32, kind="ExternalInput")
    hT_d = nc.dram_tensor("hT", [NPAIR * 128, SEQ], bf16, kind="ExternalInput")
    out_d = nc.dram_tensor("out", [NPAIR * 128, SEQ], bf16, kind="ExternalOutput")

    def dram_ap(t, offset, dims):
        return bass.AP(
            tensor=t[:, :].tensor, offset=offset, ap=[list(d) for d in dims]
        )

    with ExitStack() as ctx:
        tc = ctx.enter_context(TileContext(nc))
        const = ctx.enter_context(tc.tile_pool(name="const", bufs=1))
        persist = ctx.enter_context(tc.tile_pool(name="persist", bufs=1))
        hstp = ctx.enter_context(tc.tile_pool(name="hst", bufs=2))
        work = ctx.enter_context(tc.tile_pool(name="work", bufs=4))
        ps_s = ctx.enter_context(tc.tile_pool(name="ps_s", bufs=2, space="PSUM"))
        ps_x = ctx.enter_context(tc.tile_pool(name="ps_x", bufs=2, space="PSUM"))
        ps_c = ctx.enter_context(tc.tile_pool(name="ps_c", bufs=4, space="PSUM"))

        bd_s = const.tile([SUB, 128], bf16)
        nc.sync.dma_start(out=bd_s, in_=bd_d[:, :])
        gw_s = const.tile([128, C_OUT], bf16)
        nc.sync.dma_start(out=gw_s, in_=gw_d[:, :])
        cw_s = const.tile([128, KT * C_OUT], bf16)
        nc.sync.dma_start(out=cw_s, in_=cw_d[:, :])
        gb_s = const.tile([128, 1], f32)
        nc.sync.dma_start(out=gb_s, in_=gb_d[:, :])
        bnb_s = const.tile([128, 1], f32)
        nc.sync.dma_start(out=bnb_s, in_=bnb_d[:, :])

        xpads = []
        for i in range(2):
            xp = persist.tile([128, XCOLS], bf16, tag=f"xpad{i}")
            nc.vector.memset(xp[:, 0:LPAD], 0.0)
            nc.vector.memset(xp[:, LPAD + SEQ : XCOLS], 0.0)
            xpads.append(xp)

        for rep in range(reps):
          for pair in range(NPAIR):
            xp = xpads[pair]
            # h staging, bf16 (cast during SWDGE DMA):
            # [u(125), m(15), j(4), S(2), c(64)] — per-(m,j) subtile is the
            # contiguous 128-col matmul stationary; per-(m) slice is the
            # residual operand.
            hb2 = hstp.tile([SUB, NMAC, NSUB, 2, C_IN], bf16, tag="hb2")
            # feature-major staging: [128 (Sslot,c), node] bf16; Sslot halves
            # are seq-swapped for odd chunks (matches crossed conv quadrants)
            hT_t = hstp.tile([128, NMAC, MAC], bf16, tag="hT")
            out_bigT = hstp.tile([128, NMAC, MAC], bf16, tag="out_bigT")
            for m0, m1 in ((0, 8), (8, NMAC)):
                nc.sync.dma_start(
                    out=hT_t[:, m0:m1, :].rearrange("p m n -> p (m n)"),
                    in_=hT_d[pair * 128 : (pair + 1) * 128, m0 * MAC : m1 * MAC],
                )
            WI = NSUB * 2 * C_IN  # cols per macro in the packed layout
            for m0, m1 in ((0, 8), (8, NMAC)):
                nc.sync.dma_start(
                    out=hb2[:, m0:m1, :, :, :].rearrange("p m j s c -> p (m j s c)"),
                    in_=h_d[pair * SUB : (pair + 1) * SUB, m0 * WI : m1 * WI],
                )

            # ---- GCN phase: 15 macro tiles, 2-deep software pipeline ----
            # agg+transpose fused: sT[(S,c), v] = sum_u h[u, (S,c)] * bd[u, v]
            sts = {}

            def emit_agg(m):
                sT_ps = ps_s.tile([128, NSUB, 128], f32, tag="sT_ps")
                for j in range(NSUB):
                    nc.tensor.matmul(
                        sT_ps[:, j, :],
                        hb2[:, m, j, :, :].rearrange("p s c -> p (s c)"),
                        bd_s,
                        start=True,
                        stop=True,
                    )
                sT_sb = work.tile([128, NSUB * 128], bf16, tag="sT_sb")
                nc.vector.tensor_copy(sT_sb, sT_ps.rearrange("p j n -> p (j n)"))
                sts[m] = sT_sb

            def emit_gw(m):
                sT_sb = sts.pop(m)
                xT_ps = ps_x.tile([128, NSUB, 128], f32, tag="xT_ps")
                xf = xT_ps.rearrange("p j n -> p (j n)")
                nc.tensor.matmul(
                    xf[0:64, :], gw_s[0:64, :], sT_sb[0:64, :],
                    start=True, stop=True,
                )
                nc.tensor.matmul(
                    xf[64:128, :], gw_s[64:128, :], sT_sb[64:128, :],
                    start=True, stop=True,
                )
                xdst = xp[:, LPAD + m * MAC : LPAD + (m + 1) * MAC].rearrange(
                    "p (j n) -> p j n", n=SUB
                )
                if m % 3 != 2:
                    nc.scalar.activation(xdst, xT_ps[:, :, 0:SUB], Relu, bias=gb_s)
                else:
                    nc.vector.tensor_scalar(
                        xdst, xT_ps[:, :, 0:SUB], gb_s, 0.0,
                        mybir.AluOpType.add, mybir.AluOpType.max,
                    )

            for m in range(NMAC):
                emit_agg(m)
                if m >= 2:
                    emit_gw(m - 2)
            emit_gw(NMAC - 2)
            emit_gw(NMAC - 1)

            # ---- conv + BN + ReLU + residual: chunk-pair groups, 1-deep
            # software pipeline so group g's drains hide under g+1's matmuls.
            groups = [
                (cb,) if cb + 1 >= NMAC else (cb, cb + 1)
                for cb in range(0, NMAC, 2)
            ]
            pend = None

            def emit_conv(chunks):
                cps = {
                    ci: ps_c.tile([128, 512], f32, tag="cps", name=f"cps_{pair}_{ci}")[
                        :, 0:MAC
                    ]
                    for ci in chunks
                }
                for k in range(KT):
                    st, sp = (k == 0), (k == KT - 1)
                    wlo = cw_s[0:64, k * C_OUT : (k + 1) * C_OUT]
                    whi = cw_s[64:128, k * C_OUT : (k + 1) * C_OUT]
                    for ci in chunks:
                        r = xp[:, ci * MAC + k * V : ci * MAC + k * V + MAC]
                        # skip_group_check: the sim's zero-region tracker
                        # mis-flattens partition-sliced bank views; the
                        # quadrant-packed accumulation pattern is HW-valid.
                        if ci % 2 == 0:  # seq0 -> top, seq1 -> bottom
                            nc.tensor.matmul(cps[ci][0:64, :], wlo, r[0:64, :], start=st, stop=sp, skip_group_check=True)
                            nc.tensor.matmul(cps[ci][64:128, :], whi, r[64:128, :], start=st, stop=sp, skip_group_check=True)
                        else:  # crossed quadrants: seq0 -> bottom, seq1 -> top
                            nc.tensor.matmul(cps[ci][64:128, :], wlo, r[0:64, :], start=st, stop=sp, skip_group_check=True)
                            nc.tensor.matmul(cps[ci][0:64, :], whi, r[64:128, :], start=st, stop=sp, skip_group_check=True)
                return cps

            def emit_drain(cps):
                # one yst tile spans the whole group so the residual add
                # runs as a single wide GpSimd op per group (halves the
                # per-op overhead on Pool)
                cis = sorted(cps.keys())
                c0, cn = cis[0], len(cis)
                yst = work.tile([128, 2, 512], bf16, tag="yst")
                for k, ci in enumerate(cis):
                    if ci % 3 != 2:
                        nc.scalar.activation(
                            yst[:, k, 0:MAC], cps[ci], Relu, bias=bnb_s
                        )
                    else:
                        nc.vector.tensor_scalar(
                            yst[:, k, 0:MAC], cps[ci], bnb_s, 0.0,
                            mybir.AluOpType.add, mybir.AluOpType.max,
                        )
                # residual in feature-major on the idle GpSimd engine;
                # hT_t rows already seq-swapped for odd chunks
                nc.gpsimd.tensor_add(
                    out_bigT[:, c0 : c0 + cn, :],
                    yst[:, 0:cn, 0:MAC],
                    hT_t[:, c0 : c0 + cn, :],
                )

            for g in groups:
                cps = emit_conv(g)
                if pend is not None:
                    emit_drain(pend)
                pend = cps
            emit_drain(pend)

            for m0, m1 in ((0, 8), (8, NMAC)):
                nc.scalar.dma_start(
                    out=out_d[pair * 128 : (pair + 1) * 128, m0 * MAC : m1 * MAC],
                    in_=out_bigT[:, m0:m1, :].rearrange("p m n -> p (m n)"),
                )

    nc.compile()
    return nc


def _consts(adj, gcn_w, gcn_b, conv_w, conv_b, bn_gamma, bn_beta, bn_mean, bn_var):
    adj = np.asarray(adj, np.float32)
    norm = adj.sum(axis=1) ** -0.5
    an = (norm[:, None] * adj * norm[None, :]).astype(np.float32)
    bd = np.zeros((SUB, 128), np.float32)
    for g in range(SUB // V):
        bd[g * V : (g + 1) * V, g * V : (g + 1) * V] = an

    gcn_w = np.asarray(gcn_w, np.float32)
    gw = np.concatenate([gcn_w, gcn_w], axis=0)  # (128, 64), rows 64:128 duplicate

    inv_std = np.asarray(bn_gamma, np.float32) / np.sqrt(
        np.asarray(bn_var, np.float32) + BN_EPS
    )
    conv_w = np.asarray(conv_w, np.float32)  # (O, I, KT, 1)
    cw = np.zeros((128, KT * C_OUT), np.float32)
    for k in range(KT):
        # BN scale folded into the conv weights (per output channel)
        wkT = conv_w[:, :, k, 0].T * inv_std[None, :]  # (I, O)
        cw[0:64, k * C_OUT : (k + 1) * C_OUT] = wkT
        cw[64:128, k * C_OUT : (k + 1) * C_OUT] = wkT

    gb = np.tile(np.asarray(gcn_b, np.float32).reshape(C_OUT, 1), (2, 1))
    bnb1 = (
        (np.asarray(conv_b, np.float32) - np.asarray(bn_mean, np.float32)) * inv_std
        + np.asarray(bn_beta, np.float32)
    )
    bnb = np.tile(bnb1.reshape(C_OUT, 1), (2, 1))

    return {
        "bd": np.ascontiguousarray(bd.astype(_BF16)),
        "gw": np.ascontiguousarray(gw.astype(_BF16)),
        "cw": np.ascontiguousarray(cw.astype(_BF16)),
        "gb": np.ascontiguousarray(gb),
        "bnb": np.ascontiguousarray(bnb),
    }


def _get_nc(reps=1):
    key = ("nc", reps)
    if key not in _CACHE:
        _CACHE[key] = _build_nc(reps)
    return _CACHE[key]


def _get_exec(reps=1):
    """Compile once; return (sharded_fn, in_names, out_names, mesh_sharding,
    zero_out_shapes). The sharded fn takes globally-concatenated inputs
    (n_cores*dim0) and donated zero-init output buffers."""
    key = ("exec", reps)
    if key in _CACHE:
        return _CACHE[key]
    import jax
    from jax.sharding import Mesh, PartitionSpec, NamedSharding
    from jax.experimental.shard_map import shard_map
    from concourse import mybir
    from concourse.bass2jax import (
        _bass_exec_p,
        partition_id_tensor,
        install_neuronx_cc_hook,
    )

    install_neuronx_cc_hook()
    nc = _get_nc(reps)

    in_names, out_names, out_avals, out_shapes = [], [], [], []
    for alloc in nc.m.functions[0].allocations:
        if not isinstance(alloc, mybir.MemoryLocationSet):
            continue
        name = alloc.memorylocations[0].name
        if alloc.kind == "ExternalInput":
            if nc.partition_id_tensor is None or name != nc.partition_id_tensor.name:
                in_names.append(name)
        elif alloc.kind == "ExternalOutput":
            out_names.append(name)
            np_dt = mybir.dt.np(alloc.dtype)
            out_avals.append(
                jax.core.ShapedArray(tuple(alloc.tensor_shape), np_dt)
            )
            out_shapes.append((tuple(alloc.tensor_shape), np_dt))

    n_params = len(in_names)
    n_outs = len(out_names)
    all_in_names = list(in_names) + list(out_names)
    if nc.partition_id_tensor is not None:
        all_in_names.append(nc.partition_id_tensor.name)

    def _body(*args):
        operands = list(args)
        if nc.partition_id_tensor is not None:
            operands.append(partition_id_tensor())
        return tuple(
            _bass_exec_p.bind(
                *operands,
                out_avals=tuple(out_avals),
                in_names=tuple(all_in_names),
                out_names=tuple(out_names),
                lowering_input_output_aliases=(),
                sim_require_finite=True,
                sim_require_nnan=True,
                nc=nc,
            )
        )

    devices = jax.devices()[:NCORES]
    mesh = Mesh(np.asarray(devices), ("core",))
    sharding = NamedSharding(mesh, PartitionSpec("core"))
    donate = tuple(range(n_params, n_params + n_outs))
    sharded = jax.jit(
        shard_map(
            _body,
            mesh=mesh,
            in_specs=(PartitionSpec("core"),) * (n_params + n_outs),
            out_specs=(PartitionSpec("core"),) * n_outs,
            check_rep=False,
        ),
        donate_argnums=donate,
        keep_unused=True,
    )
    _CACHE[key] = (sharded, in_names, out_names, sharding, out_shapes)
    return _CACHE[key]


def _pack_h(h):
    """(240000, 64) f32 -> packed bf16 [(core pair u), (m j S c)]."""
    h = np.asarray(h, np.float32).reshape(NCORES, NPAIR, 2, NMAC, NSUB, SUB, C_IN)
    hp = h.transpose(0, 1, 5, 3, 4, 2, 6)  # [core, pair, u, m, j, S, c]
    return np.ascontiguousarray(
        hp.reshape(NCORES * NPAIR * SUB, NMAC * NSUB * 2 * C_IN).astype(_BF16)
    )


def _pack_hT(h):
    """(240000, 64) f32 -> feature-major bf16 [(core pair Sslot c), (ci n)],
    with the Sslot halves seq-swapped for odd chunks ci (the crossed conv
    quadrants write seq0 to the bottom PSUM half there)."""
    h = np.asarray(h, np.float32).reshape(NCORES, NPAIR, 2, NMAC, MAC, C_IN)
    hT = np.ascontiguousarray(h.transpose(0, 1, 2, 5, 3, 4))
    # [core, pair, S, c, ci, n]
    hT[:, :, :, :, 1::2, :] = hT[:, :, ::-1, :, 1::2, :].copy()
    return np.ascontiguousarray(
        hT.reshape(NCORES * NPAIR * 128, SEQ).astype(_BF16)
    )


def _unpack_out(out):
    """feature-major bf16 [(core pair Sslot c), (ci n)] -> (N, M, T, V, C)."""
    o = np.asarray(out, np.float32).reshape(NCORES, NPAIR, 2, C_OUT, NMAC, MAC)
    o[:, :, :, :, 1::2, :] = o[:, :, ::-1, :, 1::2, :].copy()  # un-swap odd chunks
    o = o.transpose(0, 1, 2, 4, 5, 3)  # [core, pair, S, ci, n, c]
    return np.ascontiguousarray(o.reshape(N, M, T, V, C_OUT))


def _global_inputs(h, consts):
    glob = {}
    for k, v in consts.items():
        glob[k] = np.concatenate([v] * NCORES, axis=0)
    glob["h"] = _pack_h(h)
    glob["hT"] = _pack_hT(h)
    return glob


def _run(h, consts):
    import jax

    sharded, in_names, out_names, sharding, out_shapes = _get_exec()
    glob = _global_inputs(h, consts)
    dev_in = [jax.device_put(glob[nm], sharding) for nm in in_names]
    zeros = [
        np.zeros((NCORES * shp[0], *shp[1:]), dt) for (shp, dt) in out_shapes
    ]
    outs = sharded(*dev_in, *zeros)
    out = np.asarray(outs[out_names.index("out")])
    return out


def _timed_run(h, consts, iters=20, reps=1):
    """Amortized per-dispatch wall time (ns) for the `reps`-repeat NEFF
    variant: inputs stay device-resident; successive dispatches are chained
    through donated output buffers."""
    import time
    import jax

    sharded, in_names, out_names, sharding, out_shapes = _get_exec(reps)
    glob = _global_inputs(h, consts)
    dev_in = [jax.device_put(glob[nm], sharding) for nm in in_names]
    zeros = [
        np.zeros((NCORES * shp[0], *shp[1:]), dt) for (shp, dt) in out_shapes
    ]
    outs = sharded(*dev_in, *zeros)
    jax.block_until_ready(outs)
    for _ in range(5):  # warm-up
        outs = sharded(*dev_in, *outs)
    jax.block_until_ready(outs)
    t0 = time.perf_counter()
    for _ in range(iters):
        outs = sharded(*dev_in, *outs)
    jax.block_until_ready(outs)
    t1 = time.perf_counter()
    return (t1 - t0) / iters * 1e9


def measure_hw_ns(h, consts, iters=50, r_hi=33, trials=2):
    """Device execution time per kernel run, measured as the slope of
    per-dispatch wall time between a 1-repeat and an r_hi-repeat NEFF of the
    identical kernel body. The fixed axon/jax dispatch cost cancels in the
    difference; what remains is hardware execution time of (r_hi - 1)
    additional kernel executions."""
    best1 = min(_timed_run(h, consts, iters=iters, reps=1) for _ in range(trials))
    besth = min(_timed_run(h, consts, iters=iters, reps=r_hi) for _ in range(trials))
    slope = (besth - best1) / (r_hi - 1)
    if slope <= 0:
        slope = best1  # fallback: overhead noise swamped the difference
    return slope, best1, besth


def kernel(h, adj, gcn_w, gcn_b, conv_w, conv_b, bn_gamma, bn_beta, bn_mean, bn_var):
    consts = _consts(
        adj, gcn_w, gcn_b, conv_w, conv_b, bn_gamma, bn_beta, bn_mean, bn_var
    )
    out = _run(h, consts)
    return _unpack_out(out)
